# revision 42
# baseline (speedup 1.0000x reference)
"""Trainium2 Bass kernel for BasicAttention — host-pretransposed operands,
f32r matmuls, software-pipelined softmax.

  proj  = keys @ W.T                    (B, NK, DV)
  L     = proj @ values.T               (B, NK, NV)
  A     = softmax(L + mask_bias, -1)
  out   = A @ values                    (B, NK, DV)

Sharding: pure data-parallel over batch — B=16 across 8 cores, 2 batches
per core, no collectives.

Layout strategy: kernel() pre-transposes keys/values/W on the host and
ships keysT [dk, nk], valuesT [dv, nv], WT [dk, dv] (all f32r) plus a
pre-masked fp16 natural-layout values copy.  The device then DMAs every
matmul operand directly into its consumption layout: NO PE identity-
matmul transposes, no stage tiles, no PSUM->SBUF transpose drains.  This
removes ~145k PE cycles/batch (keys^T+values^T+W^T transposes) that the
previous version spent on the PE, leaving the PE stream pure matmul:
proj 131k + logits 262k + out 262k = 655k cycles/batch, x2 batches
= 1.31M cycles ~= 546us @ 2.4GHz -> the roofline for f32r precision.

Precision: proj and logits matmuls run in float32r (PE reduced-precision
4-byte mode, ~13 effective mantissa bits, 1 cycle/row at moving dim >=
256).  Output stage (E, V) is fp16.  fp8/DoubleRow was analyzed and
rejected: logits absolute errors get exp-amplified (abs logit err ~1.0
at fp8 operand precision vs the ~0.07 the 2e-2 gate allows), and fp8
values would put their ~2^-4 quantization directly in the output.

Main-loop software pipelining (unchanged from the transpose version):
output matmuls for k-tile i lag the logits matmuls by two k-tiles so
each tile's softmax chain (reduce_max -> Exp -> E^T XBAR transpose)
hides under two full L/O rounds of PE work.  keys^T loads run 2
k-blocks ahead and proj 2 ahead inside the k-tile stream (continuing
across batches).  vT is held as 4 independent single-buffered bank
tiles so the next batch's bank-nb load can start as soon as the last
k-tile's bank-nb logits matmuls retire, overlapping the reload with the
out-matmul pipeline drain; vh(b+1) loads in the 2-k-tile out lag.

PSUM: pp 2 banks, L 4, O 2.
"""

import sys
import numpy as np

_TRN_REPO = "/opt/trn_rl_repo"
if _TRN_REPO not in sys.path:
    sys.path.insert(0, _TRN_REPO)

B, NK, NV, DK, DV = 16, 2048, 2048, 1024, 1024
N_CORES = 8
B_LOC = B // N_CORES
P = 128


def build_kernel(b_loc=B_LOC, nk=NK, nv=NV, dk=DK, dv=DV, loop_iters=1,
                 dummy_io=False, unroll_iters=1, n_cores=N_CORES,
                 debug_taps=0, et_queue="act", hoist_v=False,
                 load_queue="sync", et_bufs=2, emit_delay=4, et_single=True,
                 drain_engine="vector", kt_queue="gpsimd", coalesce=True):
    """Build the single-core Bass program (SPMD-replicated across 8 cores)."""
    import concourse.bacc as bacc
    import concourse.mybir as mybir
    from concourse import tile

    f32 = mybir.dt.float32
    f32r = mybir.dt.float32r
    f16 = mybir.dt.float16
    Exp = mybir.ActivationFunctionType.Exp
    Copy = mybir.ActivationFunctionType.Copy
    X = mybir.AxisListType.X

    KT, NT, DT, VT = nk // P, nv // P, dk // P, dv // P
    KB = min(256, nk)             # proj k-block (moving-dim of proj matmuls)
    KBT = KB // P                 # k-tiles per k-block
    NKB = nk // KB
    LB = min(512, nv)             # logits matmul free-dim block (1 PSUM bank)
    NB = nv // LB
    TPB = LB // P                 # n-tiles per logits bank
    OB = min(512, dv)             # output matmul free-dim block
    OBN = dv // OB

    nc = bacc.Bacc("TRN2", target_bir_lowering=False, debug=False,
                   num_devices=n_cores)
    if debug_taps:
        dbg_Et = nc.declare_dram_parameter("dbg_Et", [debug_taps, P, nv], f16,
                                           isOutput=True)
        dbg_eT = nc.declare_dram_parameter("dbg_eT", [debug_taps, P, NT * P],
                                           f16, isOutput=True)
        dbg_s = nc.declare_dram_parameter("dbg_s", [debug_taps, P, 1], f32,
                                          isOutput=True)
        dbg_mx = nc.declare_dram_parameter("dbg_mx", [debug_taps, P, NB], f32,
                                           isOutput=True)
    if dummy_io:
        # timing-only variant: big tensors live in internal DRAM scratch so
        # nothing heavy ships over the axon tunnel; compute is identical
        keysT_d = nc.dram_tensor("keysT_s", [b_loc, dk, nk], f32r)
        valuesT_d = nc.dram_tensor("valuesT_s", [b_loc, dv, nv], f32r)
        vh_d = nc.dram_tensor("vh_s", [b_loc, nv, dv], f16)
        wT_d = nc.dram_tensor("WT_s", [dk, dv], f32r)
        out_d = nc.dram_tensor("out_s", [b_loc, nk, dv], f32)
        tok_d = nc.declare_dram_parameter("tok", [1, 1], f32, isOutput=True)
    else:
        keysT_d = nc.declare_dram_parameter("keysT", [b_loc, dk, nk], f32r,
                                            isOutput=False)
        valuesT_d = nc.declare_dram_parameter("valuesT", [b_loc, dv, nv], f32r,
                                              isOutput=False)
        vh_d = nc.declare_dram_parameter("vh", [b_loc, nv, dv], f16,
                                         isOutput=False)
        wT_d = nc.declare_dram_parameter("WT", [dk, dv], f32r, isOutput=False)
        out_d = nc.declare_dram_parameter("out", [b_loc, nk, dv], f32,
                                          isOutput=True)
        tok_d = None

    with tile.TileContext(nc) as tc:
        with (
            tc.tile_pool(name="wt", bufs=1) as wtp,
            tc.tile_pool(name="vt", bufs=1) as vtp,
            tc.tile_pool(name="vhp", bufs=1) as vhp,
            tc.tile_pool(name="ktp", bufs=2) as ktp,
            tc.tile_pool(name="ptp", bufs=3) as ptp,
            tc.tile_pool(name="smp", bufs=et_bufs) as smp,
            tc.tile_pool(name="etp", bufs=max(3, emit_delay + 1)) as etp,
            tc.tile_pool(name="otp", bufs=2) as otp,
            tc.tile_pool(name="stats", bufs=6) as stp,
            tc.tile_pool(name="ps_S", bufs=2, space="PSUM") as ps_S,
            tc.tile_pool(name="ps_L", bufs=1, space="PSUM") as ps_L,
            tc.tile_pool(name="ps_O", bufs=1, space="PSUM") as ps_O,
        ):
            # W^T f32r resident across loop iterations: wt[p, dt, v] =
            # W[v, dt*128+p] = WT[dt*128+p, v]
            wt = wtp.tile([P, DT, dv], f32r, tag="wt")
            nc.sync.dma_start(wt, wT_d.rearrange("(t p) v -> p t v", p=P))

            et_dma = nc.scalar if et_queue == "act" else nc.sync
            ld_dma = nc.gpsimd if load_queue == "gpsimd" else nc.sync
            kt_dma = nc.gpsimd if kt_queue == "gpsimd" else ld_dma
            # timing ablation: load only a fraction of V (dummy data anyway)
            v_vt, v_nt = (1, 2) if hoist_v else (VT, NT)

            def load_vbank(b, nb_i):
                vb = vtp.tile([P, VT, LB], f32r, tag=f"vT{nb_i}")
                if coalesce:
                    src = valuesT_d[b].rearrange("(t p) n -> p t n", p=P)
                    ld_dma.dma_start(vb[:, :v_vt, :],
                                     src[:, :v_vt, nb_i * LB:(nb_i + 1) * LB])
                else:
                    for vt_i in range(v_vt):
                        ld_dma.dma_start(
                            vb[:, vt_i, :],
                            valuesT_d[b, vt_i * P:(vt_i + 1) * P,
                                      nb_i * LB:(nb_i + 1) * LB])
                return vb

            def load_vh(b):
                vh = vhp.tile([P, NT, dv], f16, tag="vh")
                if coalesce:
                    src = vh_d[b].rearrange("(t p) v -> p t v", p=P)
                    ld_dma.dma_start(vh[:, :v_nt, :], src[:, :v_nt, :])
                else:
                    for nt_i in range(v_nt):
                        ld_dma.dma_start(vh[:, nt_i, :],
                                         vh_d[b, nt_i * P:(nt_i + 1) * P, :])
                return vh

            def body(_i=None):
                def load_kT(b, kb_i):
                    """keys^T f32r for one k-block, straight from DRAM."""
                    kT = ktp.tile([P, DT, KB], f32r, tag="kT")
                    c0 = kb_i * KB
                    if coalesce:
                        src = keysT_d[b].rearrange("(t p) k -> p t k", p=P)
                        kt_dma.dma_start(kT, src[:, :, c0:c0 + KB])
                    else:
                        for dt_i in range(DT):
                            kt_dma.dma_start(
                                kT[:, dt_i, :],
                                keysT_d[b, dt_i * P:(dt_i + 1) * P,
                                        c0:c0 + KB])
                    return kT

                def proj_mm(kT):
                    """single-pass f32r proj matmuls -> projT."""
                    pT = ptp.tile([P, VT, KB], f32r, tag="pT")
                    for vt_i in range(0, VT, 2):
                        pp = ps_S.tile([P, 2 * KB], f32, tag="ps")
                        for h in range(2):
                            for dt_i in range(DT):
                                nc.tensor.matmul(
                                    pp[:, h * KB:(h + 1) * KB],
                                    lhsT=wt[:, dt_i,
                                            (vt_i + h) * P:(vt_i + h + 1) * P],
                                    rhs=kT[:, dt_i, :],
                                    start=(dt_i == 0), stop=(dt_i == DT - 1))
                        if drain_engine == "scalar":
                            nc.scalar.copy(
                                pT[:, vt_i:vt_i + 2, :],
                                pp.rearrange("p (a b) -> p a b", a=2))
                        else:
                            nc.vector.tensor_copy(
                                pT[:, vt_i:vt_i + 2, :],
                                pp.rearrange("p (a b) -> p a b", a=2))
                    return pT

                # global stream of k-blocks across batches: keys^T loads and
                # proj matmuls for batch b+1 run during batch b's k-loop tail
                # so the PE never starves at batch boundaries
                kblocks = [(b, kb) for b in range(b_loc) for kb in range(NKB)]
                next_load = [0]
                kTs, pTs = [], []

                def prep_load():
                    if next_load[0] < len(kblocks):
                        bb, kb = kblocks[next_load[0]]
                        next_load[0] += 1
                        kTs.append(load_kT(bb, kb))

                def prep_proj():
                    if kTs:
                        pTs.append(proj_mm(kTs.pop(0)))

                for b in range(b_loc):
                    # V^T f32r (logits rhs) as 4 single-buffered bank tiles:
                    # vbank[nb][p, vt, j] = values[b, nb*512+j, vt*128+p].
                    # DMA issue order at body start: kT0, vb0 (the two the PE
                    # needs first), then the rest, then vh (needed at the
                    # first out-emit, delayed by emit_delay tiles).
                    if b == 0:
                        prep_load()
                        vbanks = [load_vbank(b, 0)]
                        prep_load()
                        vbanks += [load_vbank(b, nb_i) for nb_i in range(1, NB)]
                        vh = load_vh(b)
                        # prime the proj pipeline two k-blocks deep
                        prep_proj()
                        prep_load()
                        prep_proj()
                    else:
                        vbanks = [load_vbank(b, nb_i) for nb_i in range(NB)]
                        vh = load_vh(b)

                    pending = []  # [(eT, r, kt_i)] awaiting output matmuls

                    def emit_output(p, vh=vh, b=b):
                        eT, r, kt_i = p
                        Op = ps_O.tile([P, dv], f32, tag="O")
                        for ob_i in range(OBN):
                            Os = Op[:, ob_i * OB:(ob_i + 1) * OB]
                            for nt_i in range(NT):
                                nc.tensor.matmul(
                                    Os, lhsT=eT[:, nt_i, :],
                                    rhs=vh[:, nt_i, ob_i * OB:(ob_i + 1) * OB],
                                    start=(nt_i == 0), stop=(nt_i == NT - 1))
                        Ot = otp.tile([P, dv], f32, tag="Ot")
                        nc.scalar.activation(Ot, Op, Copy, scale=r)
                        if coalesce:
                            nc.sync.dma_start(
                                out_d[b, kt_i * P:(kt_i + 1) * P, :], Ot)
                        else:
                            oh = dv // 2
                            nc.sync.dma_start(
                                out_d[b, kt_i * P:(kt_i + 1) * P, :oh],
                                Ot[:, :oh])
                            nc.sync.dma_start(
                                out_d[b, kt_i * P:(kt_i + 1) * P, oh:],
                                Ot[:, oh:])

                    for kb_i in range(NKB):
                        pT = pTs.pop(0)

                        for kk in range(KBT):
                            kt_i = kb_i * KBT + kk
                            ks = slice(kk * P, (kk + 1) * P)

                            # L[k, n] single-pass f32r; per-bank max as each
                            # 512-wide PSUM bank's accumulation closes
                            Lp = ps_L.tile([P, nv], f32, tag="L")
                            mx = stp.tile([P, NB], f32, tag="mx")
                            for nb_i in range(NB):
                                Ls = Lp[:, nb_i * LB:(nb_i + 1) * LB]
                                for vt_i in range(VT):
                                    nc.tensor.matmul(
                                        Ls, lhsT=pT[:, vt_i, ks],
                                        rhs=vbanks[nb_i][:, vt_i, :],
                                        start=(vt_i == 0), stop=(vt_i == VT - 1))
                                nc.vector.reduce_max(mx[:, nb_i:nb_i + 1], Ls,
                                                     axis=X)

                            # softmax: E = exp(L - max) fp16, all banks on Act
                            # first (frees Lp for L(kt+1) fast), then the E^T
                            # XBAR transposes on the SP DMA queue
                            negm = stp.tile([P, 1], f32, tag="negm")
                            nc.vector.reduce_max(negm, mx, axis=X, negate=True)
                            Et = smp.tile([P, nv], f16, tag="E")
                            s4 = stp.tile([P, NB], f32, tag="s4")
                            eT = etp.tile([P, NT, P], f16, tag="eT")
                            for nb_i in range(NB):
                                sl = slice(nb_i * LB, (nb_i + 1) * LB)
                                nc.scalar.activation(
                                    Et[:, sl], Lp[:, sl], Exp, bias=negm,
                                    scale=1.0,
                                    accum_out=s4[:, nb_i:nb_i + 1])
                            if et_single:
                                et_dma.dma_start(eT, Et, transpose=True)
                            else:
                                for nb_i in range(NB):
                                    sl = slice(nb_i * LB, (nb_i + 1) * LB)
                                    et_dma.dma_start(
                                        eT[:, nb_i * TPB:(nb_i + 1) * TPB, :],
                                        Et[:, sl], transpose=True)
                            s = stp.tile([P, 1], f32, tag="s")
                            nc.vector.reduce_sum(s, s4, axis=X)
                            r = stp.tile([P, 1], f32, tag="r")
                            nc.vector.reciprocal(r, s)

                            if debug_taps and b == 0 and kt_i < debug_taps:
                                nc.sync.dma_start(dbg_Et[kt_i], Et)
                                nc.sync.dma_start(
                                    dbg_eT[kt_i],
                                    eT.rearrange("p a b -> p (a b)"))
                                nc.sync.dma_start(dbg_s[kt_i], s)
                                nc.sync.dma_start(dbg_mx[kt_i], mx)

                            # O(kt-2) issued here: its softmax chain had two
                            # full L/O rounds of PE work to hide under.  The
                            # first emit_delay tiles of a batch hold their
                            # emits back so vh(b) has a longer DMA window.
                            pending.append((eT, r, kt_i))
                            if kt_i >= emit_delay:
                                npop = min(2, len(pending) - 2)
                                for _ in range(max(0, npop)):
                                    emit_output(pending.pop(0))

                            if kk == 0:
                                prep_load()
                            elif kk == KBT - 1 and (kb_i + 2 < NKB
                                                    or b + 1 < b_loc):
                                prep_proj()

                    for p in pending:
                        emit_output(p)

            if loop_iters > 1:
                with tc.For_i(0, loop_iters, 1):
                    body()
            else:
                for _ in range(unroll_iters):
                    body()
            if tok_d is not None:
                tok = stp.tile([1, 1], f32, tag="tok")
                nc.vector.memset(tok, 0.0)
                nc.sync.dma_start(tok_d[:, :], tok)

    nc.finalize()
    return nc


_NC_CACHE = {}


def _get_nc(**kwargs):
    key = tuple(sorted(kwargs.items()))
    if key not in _NC_CACHE:
        _NC_CACHE[key] = build_kernel(**kwargs)
    return _NC_CACHE[key]


def run(inputs, loop_iters=1, **build_kwargs):
    """Shard full inputs over the 8 cores, run, gather the full output.

    Host-side layout prep: transpose keys/values/W into the layouts the
    device consumes (contraction dim on partitions), and fold the values
    mask into a pre-cast fp16 natural-layout values copy.
    """
    from concourse.bass_utils import run_bass_kernel_spmd

    nc = _get_nc(loop_iters=loop_iters, **build_kwargs)
    keys = np.asarray(inputs["keys"], dtype=np.float32)
    values = np.asarray(inputs["values"], dtype=np.float32)
    mask = np.asarray(inputs["values_mask"], dtype=np.int32)
    w = np.asarray(inputs["W"], dtype=np.float32)

    keysT = np.ascontiguousarray(keys.transpose(0, 2, 1))          # [B, DK, NK]
    valuesT = np.ascontiguousarray(values.transpose(0, 2, 1))      # [B, DV, NV]
    vh16 = np.ascontiguousarray(
        (values * mask[:, :, None].astype(np.float32)).astype(np.float16))
    wT = np.ascontiguousarray(w.T)                                 # [DK, DV]

    in_maps = []
    for c in range(N_CORES):
        sl = slice(c * B_LOC, (c + 1) * B_LOC)
        in_maps.append({
            "keysT": keysT[sl],
            "valuesT": valuesT[sl],
            "vh": vh16[sl],
            "WT": wT,
        })
    res = run_bass_kernel_spmd(nc, in_maps, core_ids=list(range(N_CORES)))
    return np.concatenate([res.results[c]["out"] for c in range(N_CORES)], axis=0)


def kernel(**inputs) -> np.ndarray:
    return run(inputs)
